# revision 18
# baseline (speedup 1.0000x reference)
"""Trainium2 Bass kernel: single attention head (B=8, S=2048, E=1024, H=64).

Sharding: data-parallel over batch -- each of the 8 NeuronCores computes one
batch element's full attention. No collectives; every HBM byte read once.

v4: column-blocked streaming pipeline, dense PE schedule.
  - Inputs staged host-side as fp16 quarter-slabs [NB, 128, EC*BW]:
    slab[q][p][c*BW+s] = x[q*BW+s, c*128+p].  Each input DMA is a flat 2D
    pattern (8KB contiguous per partition) -> cheap HWDGE trigger.
  - One DMA priority stream on the sync ring: all xk quarters early (kt
    gates every exp), xq blocks next at consumption pace, xv late.
  - PE warm-up: a dozen dummy matmuls on the weight tile ramp the PE
    p-state to 2.4GHz before real data lands (the ramp is sticky on HW).
  - k projection runs quarter-by-quarter INTERLEAVED into block 0's score
    pairs (emitting it all up front would head-of-line block the first
    scores behind xk3's DMA).
  - Scores per (block, key-tile-pair): S^T[sk,sq] = kt.T @ qt into PSUM
    pair-tiles [128,2,512]; one ScalarE exp instruction covers 1024 cols.
  - AV (ones-augmented v gives softmax denominators for free) and q/v
    projections spread between score pairs; per-block finalize uses
    HWDGE transpose-DMA on the sync ring (idle after the input stream)
    + vector normalize; v^T tiles via transpose-DMA on the scalar ring.
  - PSUM: scores 2x2 + kproj 2 (block 0 era) / oa 3 (after) + proj 1 = 8.
"""

import numpy as np

import concourse.bass as bass  # noqa: F401  (engine namespaces live on nc)
import concourse.mybir as mybir
import concourse.tile as tile
from concourse import bacc
from concourse.bass_utils import run_bass_kernel_spmd

B, S, E, H = 8, 2048, 1024, 64
EC = E // 128   # contraction chunks per projection
NT = S // 128   # key tiles
NB = 4          # 512-column blocks
BW = S // NB
F16 = mybir.dt.float16
F32 = mybir.dt.float32

_CACHE = {}


def _build_nc():
    nc = bacc.Bacc(None)
    xq = nc.declare_dram_parameter("xqs", [NB, 128, EC * BW], F16, isOutput=False)
    xk = nc.declare_dram_parameter("xks", [NB, 128, EC * BW], F16, isOutput=False)
    xv = nc.declare_dram_parameter("xvs", [NB, 128, EC * BW], F16, isOutput=False)
    wq = nc.declare_dram_parameter("wq", [E, H], F16, isOutput=False)
    wk = nc.declare_dram_parameter("wk", [E, H], F16, isOutput=False)
    wv = nc.declare_dram_parameter("wv", [E, H], F16, isOutput=False)
    bq = nc.declare_dram_parameter("bq", [H, 1], F32, isOutput=False)
    bv = nc.declare_dram_parameter("bv", [H, 1], F32, isOutput=False)
    out = nc.declare_dram_parameter("out", [S, H], F32, isOutput=True)

    Exp = mybir.ActivationFunctionType.Exp

    with tile.TileContext(nc) as tc:
        with tc.tile_pool(name="const", bufs=1) as const, \
             tc.tile_pool(name="xqp", bufs=4) as xqp, \
             tc.tile_pool(name="xvp", bufs=4) as xvp, \
             tc.tile_pool(name="oassb", bufs=2) as oassb, \
             tc.tile_pool(name="trsp", bufs=2) as trsp, \
             tc.tile_pool(name="osbp", bufs=2) as osbp, \
             tc.tile_pool(name="rcp", bufs=2) as rcp:

            # -- small constants at the head of the gpsimd ring --
            wts = {}
            for nm, dram in (("k", wk), ("q", wq), ("v", wv)):
                wt = const.tile([128, EC, H], F16, name=f"w{nm}")
                nc.gpsimd.dma_start(
                    out=wt[:], in_=dram[:].rearrange("(c p) h -> p c h", p=128))
                wts[nm] = wt
            bq_t = const.tile([H, 1], F32, name="bq_t")
            nc.gpsimd.dma_start(out=bq_t[:], in_=bq[:])
            bv_t = const.tile([H, 1], F32, name="bv_t")
            nc.gpsimd.dma_start(out=bv_t[:], in_=bv[:])

            kt = const.tile([64, S], F16, name="kt")
            qt = const.tile([64, S], F16, name="qt")
            vt = const.tile([64, S], F16, name="vt")
            vaug = const.tile([128, NT, 80], F16, name="vaug")
            warm = const.tile([1, 8], F16, name="warm")
            warm_in = const.tile([1, 8], F16, name="warm_in")

            nc.gpsimd.memset(vaug[:, :, 64], 1.0)
            nc.gpsimd.memset(warm_in[:], 0.0)

            # -- big input DMAs: sync ring, explicit priority order --
            xq_b, xv_q = [], []
            for t in range(NB):
                xq_b.append(xqp.tile([128, EC, BW], F16, tag="xq", name=f"xq{t}"))
                xv_q.append(xvp.tile([128, EC, BW], F16, tag="xv", name=f"xv{t}"))

            def dma_slab(dst, dram, q):
                nc.sync.dma_start(
                    out=dst[:], in_=dram[q].rearrange("p (c s) -> p c s", c=EC))

            with tc.tile_pool(name="xkp", bufs=4) as xkp:
                xk_q = []
                for t in range(NB):
                    xk_q.append(
                        xkp.tile([128, EC, BW], F16, tag="xk", name=f"xk{t}"))

                dma_slab(xk_q[0], xk, 0)
                dma_slab(xq_b[0], xq, 0)
                dma_slab(xk_q[1], xk, 1)
                dma_slab(xk_q[2], xk, 2)
                dma_slab(xk_q[3], xk, 3)
                dma_slab(xq_b[1], xq, 1)
                dma_slab(xv_q[0], xv, 0)
                dma_slab(xv_q[1], xv, 1)
                dma_slab(xq_b[2], xq, 2)
                dma_slab(xq_b[3], xq, 3)
                dma_slab(xv_q[2], xv, 2)
                dma_slab(xv_q[3], xv, 3)

                # warm the Exp activation table off the critical path
                nc.scalar.activation(warm[:], warm_in[:], Exp, scale=0.125)

                with tc.tile_pool(name="sps", bufs=2, space="PSUM") as sps, \
                     tc.tile_pool(name="pps", bufs=1, space="PSUM") as pps:

                    pts = [None] * NB
                    oas = [None] * NB
                    prj = [None]

                    def proj_mm(w, xtile, dsti, c):
                        # one filler matmul of a q/v projection (shared psum)
                        if c == 0:
                            prj[0] = pps.tile(
                                [64, BW], F32, tag="pp", name=f"pp{w}{dsti}")
                        nc.tensor.matmul(
                            prj[0][:], wts[w][:, c, :], xtile[:, c, :],
                            start=(c == 0), stop=(c == EC - 1),
                            skip_group_check=True)
                        if c == EC - 1:
                            bias = bq_t if w == "q" else bv_t
                            tgt = qt if w == "q" else vt
                            nc.vector.tensor_scalar_add(
                                tgt[:, dsti * BW:(dsti + 1) * BW],
                                prj[0][:], bias[:])

                    def qproj_mm(j, c):
                        proj_mm("q", xq_b[j], j, c)

                    def vproj_mm(q, c):
                        proj_mm("v", xv_q[q], q, c)

                    def vaug_t(q):
                        # HWDGE transpose trigger on the scalar ring
                        nc.scalar.dma_start_transpose(
                            vaug[:, 4 * q:4 * (q + 1), 0:64],
                            vt[:, q * BW:(q + 1) * BW])

                    def av_pair(j, t2):
                        for t in (t2, t2 + 1):
                            nc.tensor.matmul(
                                oas[j][:], vaug[:, t, 0:65], pts[j][:, t, :],
                                start=(t == 0), stop=(t == NT - 1),
                                skip_group_check=True)

                    def scores_pair(j, i):
                        st = sps.tile(
                            [128, 2, BW], F32, tag="st", name=f"st{j}_{i}")
                        for u in range(2):
                            nc.tensor.matmul(
                                st[:, u, :],
                                kt[:, (2 * i + u) * 128:(2 * i + u + 1) * 128],
                                qt[:, j * BW:(j + 1) * BW],
                                start=True, stop=True)
                        nc.scalar.activation(
                            pts[j][:, 2 * i:2 * i + 2, :], st[:],
                            Exp, scale=0.125)

                    def fin(j):
                        # evacuate oa, transpose on the (idle) sync ring,
                        # normalize by the row-sum column, store
                        oasb = oassb.tile(
                            [80, BW], F16, tag="oasb", name=f"oasb{j}")
                        nc.vector.tensor_copy(oasb[0:65, :], oas[j][:])
                        trs = trsp.tile(
                            [128, 4, 80], F16, tag="trs", name=f"trs{j}")
                        nc.sync.dma_start_transpose(trs[:], oasb[:])
                        osb = osbp.tile(
                            [128, 4, H], F32, tag="osb", name=f"osb{j}")
                        for jj in range(4):
                            rc = rcp.tile(
                                [128, 1], F32, tag="rc", name=f"rc{j}_{jj}")
                            nc.vector.reciprocal(rc[:], trs[:, jj, 64:65])
                            nc.vector.tensor_scalar(
                                osb[:, jj, :], trs[:, jj, 0:64], rc[:], None,
                                op0=mybir.AluOpType.mult)
                        out_r = out[:].rearrange("(t p) h -> p t h", p=128)
                        nc.gpsimd.dma_start(
                            out=out_r[:, 4 * j:4 * (j + 1), :], in_=osb[:])

                    # ---- block 0 era: kproj pool (2 banks) still open ----
                    with tc.tile_pool(name="kqp", bufs=2, space="PSUM") as kqp:

                        def kproj_q(t):
                            pk = kqp.tile(
                                [64, BW], F32, tag="kq", name=f"kq{t}")
                            for c in range(EC):
                                nc.tensor.matmul(
                                    pk[:], wts["k"][:, c, :], xk_q[t][:, c, :],
                                    start=(c == 0), stop=(c == EC - 1),
                                    skip_group_check=True)
                            nc.vector.tensor_copy(
                                kt[:, t * BW:(t + 1) * BW], pk[:])

                        # PE p-state warm-up: dummy matmuls on the weight tile
                        dmy = kqp.tile([64, BW], F32, tag="kq", name="dmy")
                        for r in range(12):
                            nc.tensor.matmul(
                                dmy[:], wts["k"][:, 0, :], wts["k"][:, :, :],
                                start=True, stop=True, skip_group_check=True)

                        kproj_q(0)
                        for c in range(EC):
                            qproj_mm(0, c)

                        pts[0] = const.tile([128, NT, BW], F16, name="pt0")
                        for i in range(NT // 2):
                            scores_pair(0, i)
                            if i == 0:
                                kproj_q(1)
                            if i == 2:
                                kproj_q(2)
                            if i == 4:
                                kproj_q(3)
                            if i in (6, 7):
                                for c in range(4 * (i - 6), 4 * (i - 6) + 4):
                                    qproj_mm(1, c)

                    # ---- blocks 1-3: oa pool (3 banks) ----
                    with tc.tile_pool(name="oap", bufs=3, space="PSUM") as oap:
                        for j in range(1, NB):
                            pts[j] = const.tile([128, NT, BW], F16,
                                                name=f"pt{j}")

                        # block 1: fillers = qproj 2, vproj 0, vproj 1
                        oas[0] = oap.tile([65, BW], F32, tag="oa", name="oa0")
                        for i in range(NT // 2):
                            scores_pair(1, i)
                            if i in (3, 4):
                                for c in range(4 * (i - 3), 4 * (i - 3) + 4):
                                    qproj_mm(2, c)
                            if i in (5, 6):
                                for c in range(4 * (i - 5), 4 * (i - 5) + 4):
                                    vproj_mm(0, c)
                                if i == 6:
                                    vaug_t(0)
                            if i == 7:
                                for c in range(8):
                                    vproj_mm(1, c)
                        vaug_t(1)

                        # block 2: AV tiles 0-7 of blocks 0/1; qproj 3,
                        # vproj 2 late (xv2 arrives mid-block), vproj 3 last
                        oas[1] = oap.tile([65, BW], F32, tag="oa", name="oa1")
                        for i in range(NT // 2):
                            scores_pair(2, i)
                            if i == 0:
                                av_pair(0, 0)
                            if i == 1:
                                av_pair(0, 2)
                            if i == 2:
                                av_pair(1, 0)
                                qproj_mm(3, 0)
                                qproj_mm(3, 1)
                                qproj_mm(3, 2)
                                qproj_mm(3, 3)
                            if i == 3:
                                av_pair(0, 4)
                                qproj_mm(3, 4)
                                qproj_mm(3, 5)
                                qproj_mm(3, 6)
                                qproj_mm(3, 7)
                            if i == 4:
                                av_pair(1, 2)
                            if i == 5:
                                av_pair(0, 6)
                                for c in range(4):
                                    vproj_mm(2, c)
                            if i == 6:
                                av_pair(1, 4)
                                for c in range(4, 8):
                                    vproj_mm(2, c)
                            if i == 7:
                                av_pair(1, 6)
                                for c in range(8):
                                    vproj_mm(3, c)
                        vaug_t(2)

                        # block 3: AV tiles 8-15 of blocks 0/1 as vaug q2/q3
                        # land; av(3) chases exp(3) with a one-pair lag
                        oas[3] = oap.tile([65, BW], F32, tag="oa", name="oa3")
                        for i in range(NT // 2):
                            scores_pair(3, i)
                            if i == 0:
                                av_pair(0, 8)
                                vaug_t(3)
                            if i == 1:
                                av_pair(0, 10)
                                av_pair(3, 0)
                            if i == 2:
                                av_pair(1, 8)
                                av_pair(3, 2)
                            if i == 3:
                                av_pair(1, 10)
                                av_pair(3, 4)
                            if i == 4:
                                av_pair(0, 12)
                                av_pair(3, 6)
                            if i == 5:
                                av_pair(1, 12)
                                av_pair(3, 8)
                            if i == 6:
                                av_pair(0, 14)
                                av_pair(3, 10)
                            if i == 7:
                                av_pair(1, 14)
                                av_pair(3, 12)

                        # tail
                        av_pair(3, 14)
                        fin(0)
                        fin(1)
                        oas[2] = oap.tile([65, BW], F32, tag="oa", name="oa2")
                        for p in range(8):
                            av_pair(2, 2 * p)
                        fin(3)
                        fin(2)

    nc.finalize()
    return nc


def get_nc():
    if "nc" not in _CACHE:
        _CACHE["nc"] = _build_nc()
    return _CACHE["nc"]


def _slab(x):
    # [S, E] f32 -> [NB, 128, EC*BW] f16, slab[q, p, c*BW+s] = x[q*BW+s, c*128+p]
    a = x.reshape(NB, BW, EC, 128).transpose(0, 3, 2, 1).astype(np.float16)
    return np.ascontiguousarray(a.reshape(NB, 128, EC * BW))


def make_in_maps(inputs):
    q = np.asarray(inputs["query"], np.float32)
    k = np.asarray(inputs["key_"], np.float32)
    v = np.asarray(inputs["value"], np.float32)
    wq = np.ascontiguousarray(np.asarray(inputs["Wq"], np.float32).astype(np.float16))
    wk = np.ascontiguousarray(np.asarray(inputs["Wk"], np.float32).astype(np.float16))
    wv = np.ascontiguousarray(np.asarray(inputs["Wv"], np.float32).astype(np.float16))
    bq = np.ascontiguousarray(np.asarray(inputs["bq"], np.float32).reshape(H, 1))
    bv = np.ascontiguousarray(np.asarray(inputs["bv"], np.float32).reshape(H, 1))
    in_maps = []
    for b in range(B):
        in_maps.append({
            "xqs": _slab(q[b]),
            "xks": _slab(k[b]),
            "xvs": _slab(v[b]),
            "wq": wq, "wk": wk, "wv": wv,
            "bq": bq, "bv": bv,
        })
    return in_maps


def kernel(**inputs):
    nc = get_nc()
    in_maps = make_in_maps(inputs)
    res = run_bass_kernel_spmd(nc, in_maps, list(range(B)))
    return np.stack([res.results[b]["out"] for b in range(B)], axis=0)
